# revision 6
# baseline (speedup 1.0000x reference)
"""Diagonal complex SSM (LRU-style scan) on 8 trn2 NeuronCores.

y[t,p,k] = Re( C @ s[t,:,k] ) + (D @ x[t,:,k])
s[t,n,k] = A[n,k] * s[t-1,n,k] + (B @ x[t,:,k])[n]     (complex, diagonal)

Strategy: shard K=32 across 8 cores (4 lanes each; B/C/D replicated, no
collectives). Per core, linearize the complex scan by phase with
CHUNK-LOCAL tables:  within a chunk of TB steps starting at t0,
    s[t0+i] = e^{i theta i} * q[i],   q[i] = r * q[i-1] + e^{-i theta i} u[t0+i]
so the recurrence is two REAL first-order hardware scans per lane, with
FIXED [128, TB] cos/sin tables reused by every chunk (no per-chunk table
DMA).  All 16 (h,k,comp) segments scan in ONE [128, 8192] DVE op using
a bf16 multiplier mask (0 at segment starts); the scan multiplier is
rt = bf16(r) EXACTLY, with the (r/rt)^i correction folded into the fp64
host tables, so the realized pole is exact.  Carries re-enter via a
tiny packed complex mul injected into each segment's first column.
All elementwise work is bf16 (DVE 2x mode), all matmuls bf16, scan
state fp32 internal.  Rotate-out adds fold into the C matmuls via
signed stationaries on the negated-imag channel q~.
"""

import numpy as np
import ml_dtypes

from concourse import bacc, mybir
from concourse.tile import TileContext
from concourse.bass_utils import run_bass_kernel_spmd

T, N, U, K, P = 4096, 256, 128, 32, 128
NCORES = 8
KL = K // NCORES          # k-lanes per core
TB = 512                  # timesteps per chunk (table period)
NT = T // TB
F32 = mybir.dt.float32
BF16 = mybir.dt.bfloat16
BF16NP = ml_dtypes.bfloat16

_CACHE = {}

mult = mybir.AluOpType.mult
add = mybir.AluOpType.add


def _build():
    nc = bacc.Bacc("TRN2", target_bir_lowering=False, debug=False,
                   num_devices=NCORES)

    xT_d = nc.dram_tensor("xT", [U, KL, T], BF16, kind="ExternalInput")
    # fixed chunk-local tables, [n-half-part, k, comp, i]
    Ws_d = [nc.dram_tensor(f"Ws{h}", [128, KL * 2 * TB], BF16,
                           kind="ExternalInput") for h in range(2)]
    rm_d = nc.dram_tensor("rmaskA", [128, 2 * KL * 2 * TB], BF16,
                          kind="ExternalInput")
    co_d = [nc.dram_tensor(f"co{h}", [128, KL * 2 * TB], BF16,
                           kind="ExternalInput") for h in range(2)]
    cs_d = [nc.dram_tensor(f"cs{h}", [128, KL * 2 * TB], BF16,
                           kind="ExternalInput") for h in range(2)]
    # scan decay multipliers r packed [p, h*KL + k]
    r_d = nc.dram_tensor("rdec", [128, 2 * KL], F32, kind="ExternalInput")
    # carry re-base constants per h: REB1 [p, (k,c)] = cos(theta*TB),
    # REB2 [p, 0:4]=-sin(theta*TB) (k), [p,4:8]=+sin(theta*TB) (k)
    RB1_d = nc.dram_tensor("RB1A", [128, 4 * KL], F32,
                           kind="ExternalInput")
    RB2_d = nc.dram_tensor("RB2A", [128, 4 * KL], F32,
                           kind="ExternalInput")
    Bre_d = nc.dram_tensor("BTre", [U, N], BF16, kind="ExternalInput")
    Bim_d = nc.dram_tensor("BTim", [U, N], BF16, kind="ExternalInput")
    C1_d = nc.dram_tensor("CT1", [128, N], BF16, kind="ExternalInput")
    C2_d = nc.dram_tensor("CT2", [128, N], BF16, kind="ExternalInput")
    C3_d = nc.dram_tensor("CT3", [128, N], BF16, kind="ExternalInput")
    DT_d = nc.dram_tensor("DT", [U, P], BF16, kind="ExternalInput")
    y_d = nc.dram_tensor("yT", [P, KL, T], BF16, kind="ExternalOutput")

    with TileContext(nc) as tc:
        with (
            tc.tile_pool(name="const", bufs=1) as cpool,
            tc.tile_pool(name="xp", bufs=2) as xpool,
            tc.tile_pool(name="ua", bufs=1) as uapool,
            tc.tile_pool(name="pp", bufs=1) as ppool,
            tc.tile_pool(name="uh", bufs=1) as uhpool,
            tc.tile_pool(name="qq", bufs=2) as qpool,
            tc.tile_pool(name="pr", bufs=1) as prpool,
            tc.tile_pool(name="rz", bufs=2) as rzpool,
            tc.tile_pool(name="yo", bufs=2) as ypool,
            tc.tile_pool(name="ups", bufs=2, space="PSUM") as upsum,
            tc.tile_pool(name="yps", bufs=1, space="PSUM") as ypsum,
        ):
            Bre = cpool.tile([U, N], BF16)
            nc.sync.dma_start(Bre[:], Bre_d[:])
            Bim = cpool.tile([U, N], BF16)
            nc.sync.dma_start(Bim[:], Bim_d[:])
            # chunk-0 x prefetch goes right after the B stationaries so
            # the B matmuls start while the big tables stream in.
            xt0 = xpool.tile([U, KL, TB], BF16, tag="x", name="xt0")
            nc.sync.dma_start(xt0[:], xT_d[:, :, 0:TB])
            Ws, cs3, csO = [], [], []
            for h in range(2):
                t2 = cpool.tile([128, KL, 2, TB], BF16, name=f"Wst{h}", tag=f"Wst{h}")
                nc.sync.dma_start(
                    t2[:].rearrange("p k c t -> p (k c t)"), Ws_d[h][:])
                Ws.append(t2)
                t3 = cpool.tile([128, KL, 2, TB], BF16, name=f"cst{h}", tag=f"cst{h}")
                nc.sync.dma_start(
                    t3[:].rearrange("p k c t -> p (k c t)"), cs_d[h][:])
                cs3.append(t3)
            rmaskA = cpool.tile([128, 2 * KL * 2 * TB], BF16,
                                name="rmaskA", tag="rmaskA")
            nc.sync.dma_start(rmaskA[:], rm_d[:])
            for h in range(2):
                t3o = cpool.tile([128, KL, 2, TB], BF16, name=f"csot{h}", tag=f"csot{h}")
                nc.sync.dma_start(
                    t3o[:].rearrange("p k c t -> p (k c t)"), co_d[h][:])
                csO.append(t3o)
            C1 = cpool.tile([128, N], BF16)
            nc.sync.dma_start(C1[:], C1_d[:])
            C2 = cpool.tile([128, N], BF16)
            nc.sync.dma_start(C2[:], C2_d[:])
            C3 = cpool.tile([128, N], BF16)
            nc.sync.dma_start(C3[:], C3_d[:])
            DT = cpool.tile([U, P], BF16)
            nc.sync.dma_start(DT[:], DT_d[:])
            RB1A = cpool.tile([128, 4 * KL], F32, name="RB1A", tag="RB1A")
            nc.sync.dma_start(RB1A[:], RB1_d[:])
            RB2A = cpool.tile([128, 4 * KL], F32, name="RB2A", tag="RB2A")
            nc.sync.dma_start(RB2A[:], RB2_d[:])


            def b2k(ap):
                # [128, KL, 1, TB] slice -> [128, KL, 2, TB] stride-0 pair
                return ap.broadcast_to([128, KL, 2, TB])

            rz_prev = None
            for tb in range(NT):
                t0 = tb * TB
                if tb == 0:
                    xt = xt0
                else:
                    xt = xpool.tile([U, KL, TB], BF16, tag="x")
                    nc.sync.dma_start(xt[:], xT_d[:, :, t0:t0 + TB])

                # ---- B matmuls + PSUM->SBUF bf16 drain (Act) ----
                u_all = [uapool.tile([128, KL, 2, TB], BF16, tag=f"u{h}",
                                      name=f"uall{h}")
                         for h in range(2)]
                for h in range(2):
                    hs = slice(h * 128, (h + 1) * 128)
                    for c, Bst in ((0, Bre), (1, Bim)):
                        for kp in range(2):
                            u_ps = upsum.tile([128, 2, TB], F32, tag="u")
                            for kk in range(2):
                                nc.tensor.matmul(u_ps[:, kk, :], Bst[:, hs],
                                                 xt[:, 2 * kp + kk, :],
                                                 start=True, stop=True)
                            nc.scalar.copy(
                                u_all[h][:, 2 * kp:2 * kp + 2, c, :],
                                u_ps[:])

                # ---- rotate-in (DVE, bf16 2x) ----
                uhall = uhpool.tile([128, 2, KL, 2, TB], BF16, tag="uh")
                for h in range(2):
                    pA = ppool.tile([128, KL, 2, TB], BF16, tag=f"pA{h}")
                    nc.vector.tensor_mul(pA[:], cs3[h][:],
                                         b2k(u_all[h][:, :, 0:1, :]))
                    pB = ppool.tile([128, KL, 2, TB], BF16, tag=f"pB{h}")
                    nc.vector.tensor_mul(pB[:], Ws[h][:],
                                         b2k(u_all[h][:, :, 1:2, :]))
                    # uh'_re = c*u_re + s*u_im ; uh'_im = s*u_re - c*u_im
                    # (negated imag channel; signs folded into C stats)
                    nc.vector.tensor_sub(uhall[:, h], pA[:], pB[:])
                # inject carried state into each segment's first column
                if tb > 0:
                    rz4 = rz_prev[:].rearrange("p (h k c) -> p h k c", h=2,
                                               k=KL)
                    nc.vector.tensor_add(uhall[:, :, :, :, 0],
                                         uhall[:, :, :, :, 0], rz4[:])

                # ---- scans (GpSimd, fp32 state, bf16 in/out) ----
                qA = qpool.tile([128, 2, KL, 2, TB], BF16, tag="q",
                                name="qA")
                nc.vector.tensor_tensor_scan(
                    qA[:].rearrange("p h k c t -> p (h k c t)"),
                    rmaskA[:],
                    uhall[:].rearrange("p h k c t -> p (h k c t)"),
                    0.0, mult, add)

                # ---- carry re-base for next chunk (Act gather + tiny DVE) --
                if tb + 1 < NT:
                    zq = rzpool.tile([128, 4 * KL], F32, tag="zq")
                    nc.scalar.copy(
                        zq[:].rearrange("p (h k c) -> p h k c", h=2, k=KL),
                        qA[:, :, :, :, TB - 1])
                    m1 = rzpool.tile([128, 4 * KL], F32, tag="m1")
                    nc.vector.tensor_mul(m1[:], zq[:], RB1A[:])
                    m2 = rzpool.tile([128, 4 * KL], F32, tag="m2")
                    zq4 = zq[:].rearrange("p (h k c) -> p h k c", h=2, k=KL)
                    m24 = m2[:].rearrange("p (h k c) -> p h k c", h=2, k=KL)
                    rb4 = RB2A[:].rearrange("p (h k c) -> p h k c", h=2,
                                            k=KL)
                    nc.vector.tensor_mul(m24[:, :, :, 0:1], zq4[:, :, :, 1:2],
                                         rb4[:, :, :, 0:1])
                    nc.vector.tensor_mul(m24[:, :, :, 1:2], zq4[:, :, :, 0:1],
                                         rb4[:, :, :, 1:2])
                    rzt = rzpool.tile([128, 4 * KL], F32, tag="rz")
                    nc.vector.tensor_add(rzt[:], m1[:], m2[:])
                    rz_prev = rzt

                # ---- rotate-out products (GpSimd; overlaps DVE scan) ----
                ppr, qpr = [], []
                for h in range(2):
                    pt = prpool.tile([128, KL, 2, TB], BF16, tag=f"ppr{h}")
                    nc.gpsimd.tensor_mul(pt[:], csO[h][:],
                                         b2k(qA[:, h, :, 0:1, :]))
                    ppr.append(pt)
                    qt = prpool.tile([128, KL, 2, TB], BF16, tag=f"qpr{h}")
                    nc.gpsimd.tensor_mul(qt[:], csO[h][:],
                                         b2k(qA[:, h, :, 1:2, :]))
                    qpr.append(qt)

                # ---- C/D matmuls, stationary-major over both k-pairs ----
                # order: h0 products, ppr-h1, D, qpr-h1 last (Pool is slow)
                prods = [
                    (C1, 0, ppr[0], 0), (C3, 0, ppr[0], 1),
                    (C1, 1, ppr[1], 0), (C3, 1, ppr[1], 1),
                    (None, 0, None, 0),  # D @ x
                    (C1, 0, qpr[0], 1), (C2, 0, qpr[0], 0),
                    (C1, 1, qpr[1], 1), (C2, 1, qpr[1], 0),
                ]
                y_ps = [ypsum.tile([128, 2, TB], F32, tag=f"y{_kp}",
                                   name=f"yps{_kp}") for _kp in range(2)]
                nmm = len(prods)
                for i, (cst, h, pt, c) in enumerate(prods):
                    for k in range(KL):
                        yo_ = y_ps[k // 2][:, k % 2, :]
                        if cst is None:
                            nc.tensor.matmul(yo_, DT[:],
                                             xt[:, k, :],
                                             start=(i == 0),
                                             stop=(i == nmm - 1))
                        else:
                            hs = slice(h * 128, (h + 1) * 128)
                            nc.tensor.matmul(yo_, cst[:, hs],
                                             pt[:, k, c, :],
                                             start=(i == 0),
                                             stop=(i == nmm - 1))
                for kp in range(2):
                    y_sb = ypool.tile([128, 2, TB], BF16, tag="ysb")
                    nc.scalar.copy(y_sb[:], y_ps[kp][:])
                    nc.sync.dma_start(
                        y_d[:, 2 * kp:2 * kp + 2, t0:t0 + TB], y_sb[:])

    nc.compile()
    return nc


def _host_prep(input_sequence, A_re, A_im, B_re, B_im, C_re, C_im, D):
    """Build the per-core input maps (numpy only)."""
    x = np.ascontiguousarray(np.asarray(input_sequence), dtype=np.float32)
    A_re = np.asarray(A_re, dtype=np.float32)
    A_im = np.asarray(A_im, dtype=np.float32)
    B_re = np.asarray(B_re, dtype=np.float32)
    B_im = np.asarray(B_im, dtype=np.float32)
    C_re = np.asarray(C_re, dtype=np.float32)
    C_im = np.asarray(C_im, dtype=np.float32)
    D = np.asarray(D, dtype=np.float32)

    th = np.arctan2(A_im.astype(np.float64), A_re.astype(np.float64))  # (N,K)
    r = np.hypot(A_re.astype(np.float64), A_im.astype(np.float64))    # (N,K)

    i = np.arange(TB, dtype=np.float64)
    ang = th[:, :, None] * i[None, None, :]          # (N, K, TB) local phase
    cosL = np.cos(ang)
    sinL = np.sin(ang)
    angE = th * TB                                    # (N, K) re-base phase
    cosE = np.cos(angE).astype(np.float32)
    sinE = np.sin(angE).astype(np.float32)

    BTre = np.ascontiguousarray(B_re.T).astype(BF16NP)      # (U, N)
    BTim = np.ascontiguousarray(B_im.T).astype(BF16NP)
    CT1 = np.concatenate([C_re[:, :128].T, C_re[:, 128:].T], axis=1)
    CT2 = np.concatenate([C_im[:, :128].T, C_im[:, 128:].T], axis=1)
    CT3 = -CT2
    CT1 = np.ascontiguousarray(CT1).astype(BF16NP)          # (128, N)
    CT2 = np.ascontiguousarray(CT2).astype(BF16NP)
    CT3 = np.ascontiguousarray(CT3).astype(BF16NP)
    DTm = np.ascontiguousarray(D.T).astype(BF16NP)          # (U, P)

    in_maps = []
    for cidx in range(NCORES):
        ks = slice(cidx * KL, (cidx + 1) * KL)
        xT = np.ascontiguousarray(
            x[:, :, ks].transpose(1, 2, 0)).astype(BF16NP)  # (U,KL,T)
        m = dict(xT=xT, BTre=BTre, BTim=BTim, CT1=CT1, CT2=CT2,
                 CT3=CT3, DT=DTm)
        rc = r[:, ks]                                        # (N, KL)
        m["rdec"] = np.ascontiguousarray(np.concatenate(
            [rc[:128, :], rc[128:, :]], axis=1)).astype(np.float32)
        for h in range(2):
            hs = slice(h * 128, (h + 1) * 128)
            cl = cosL[hs, ks, :]                             # (128, KL, TB)
            sl = sinL[hs, ks, :]
            # exact-pole trick: the scan multiplier is rt = bf16(r)
            # EXACTLY; the tables carry the correction g[i] = (r/rt)^i
            # computed in fp64, so the realized pole is exact.
            rh = r[hs, ks]                                   # (128, KL)
            rt = np.asarray(rh, dtype=np.float32).astype(
                BF16NP).astype(np.float64)                   # bf16-exact
            lg = np.log(rh / rt)                             # (128, KL)
            g = np.exp(lg[:, :, None] * i[None, None, :])    # (128, KL, TB)
            gi = 1.0 / g
            ws = np.stack([-sl * gi, cl * gi], axis=2)
            c3 = np.stack([cl * gi, sl * gi], axis=2)
            cO = np.stack([cl * g, sl * g], axis=2)
            m[f"Ws{h}"] = np.ascontiguousarray(
                ws.reshape(128, -1)).astype(BF16NP)
            m[f"cs{h}"] = np.ascontiguousarray(
                c3.reshape(128, -1)).astype(BF16NP)
            m[f"co{h}"] = np.ascontiguousarray(
                cO.reshape(128, -1)).astype(BF16NP)
            # scan multiplier mask: rt everywhere, 0 at segment starts
            rmk = np.broadcast_to(rt[:, :, None, None],
                                  (128, KL, 2, TB)).copy()
            rmk[:, :, :, 0] = 0.0
            m.setdefault("_rmk", []).append(rmk.reshape(128, -1))
            # carry re-base: inject = rt*E''*q_last,
            # rt*E'' = e^{i theta TB} * r^TB / rt^(TB-1)
            fac = np.exp(np.log(rh) * TB - np.log(rt) * (TB - 1))
            cE = cosE[hs, ks] * fac                          # (128, KL)
            sE = sinE[hs, ks] * fac
            rb1 = np.repeat(cE, 2, axis=1)                   # (128, 2KL) (k,c)
            rb2 = np.concatenate([sE, -sE], axis=1)          # (128, 2KL)
            m.setdefault("_rb1", []).append(rb1)
            # RB2A layout: col (h,k,c): c=0 -> +sE (mult q~_im),
            #                           c=1 -> -sE (mult q~_re)
            rb2i = np.stack([sE, -sE], axis=2).reshape(128, -1)
            m.setdefault("_rb2", []).append(rb2i)
        m["rmaskA"] = np.ascontiguousarray(
            np.concatenate(m.pop("_rmk"), axis=1)).astype(BF16NP)
        m["RB1A"] = np.ascontiguousarray(
            np.concatenate(m.pop("_rb1"), axis=1)).astype(np.float32)
        m["RB2A"] = np.ascontiguousarray(
            np.concatenate(m.pop("_rb2"), axis=1)).astype(np.float32)
        in_maps.append(m)
    return in_maps


def _get_nc():
    if "nc" not in _CACHE:
        _CACHE["nc"] = _build()
    return _CACHE["nc"]


def kernel(input_sequence, A_re, A_im, B_re, B_im, C_re, C_im, D,
           trace=False):
    nc = _get_nc()
    in_maps = _host_prep(input_sequence, A_re, A_im, B_re, B_im, C_re,
                         C_im, D)
    res = run_bass_kernel_spmd(nc, in_maps, core_ids=list(range(NCORES)),
                               trace=trace)
    out = np.empty((T, P, K), dtype=np.float32)
    for c in range(NCORES):
        yT = res.results[c]["yT"]                    # (P, KL, T) bf16
        out[:, :, c * KL:(c + 1) * KL] = yT.transpose(2, 0, 1) \
            .astype(np.float32)
    if trace:
        _CACHE["exec_time_ns"] = res.exec_time_ns
    return out



# revision 8
# speedup vs baseline: 2.1832x; 2.1832x over previous
"""Diagonal complex SSM (LRU-style scan) on 8 trn2 NeuronCores — radix-2.

y[t,p,k] = Re( C @ s[t,:,k] ) + (D @ x[t,:,k])
s[t,n,k] = A[n,k] * s[t-1,n,k] + (B @ x[t,:,k])[n]     (complex, diagonal)

Strategy: shard K=32 across 8 cores (4 lanes each; B/C/D replicated, no
collectives).  The DVE scan is the bottleneck engine, so a RADIX-2
decimation halves all per-element DVE work:

  odd states  sigma[m] = s[2m+1] follow  sigma[m] = a^2 sigma[m-1] + w[m]
  with        w[m] = a*u[2m] + u[2m+1]  computed IN THE B MATMULS via
  host-folded per-k stationaries  B1 = Re(diag(a)B), B2 = Im(diag(a)B)
  (PSUM-accumulated with the plain B taps — zero DVE cost).

  The half-length scan uses the chunk-local rotation tables of the hatted
  system (theta^=2*theta, r^=r^2) with the exact-pole bf16 trick (scan
  multiplier r^t = bf16(r^2) exactly; fp64 correction (r^2/r^t)^i folded
  into the tables).

  odd outputs:  y[2m+1] = Re(C sigma[m]) + D x[2m+1]  via rotate-out
  products (csO tables) and signed C stationaries, as before.
  even outputs: y[2m+2] = Re(C a sigma[m]) + (Re(CB)+D) x[2m+2]: a second
  product set with a-premultiplied tables csOE = a*csO reuses the SAME C
  stationaries; Re(CB)+D is host-folded into one real stationary.  The
  chunk-boundary even column comes from the previous chunk's last product
  column (tiny Act copy), and y[0] = (Re(CB)+D) x[0] falls out naturally.
"""

import numpy as np
import ml_dtypes

from concourse import bacc, mybir
from concourse.tile import TileContext
from concourse.bass_utils import run_bass_kernel_spmd

T, N, U, K, P = 4096, 256, 128, 32, 128
NCORES = 8
KL = K // NCORES          # k-lanes per core
TB = 512                  # t-steps per chunk
TBH = TB // 2             # m-steps (pairs) per chunk = table period
NT = T // TB
F32 = mybir.dt.float32
BF16 = mybir.dt.bfloat16
BF16NP = ml_dtypes.bfloat16

_CACHE = {}

mult = mybir.AluOpType.mult
add = mybir.AluOpType.add


def _build():
    nc = bacc.Bacc("TRN2", target_bir_lowering=False, debug=False,
                   num_devices=NCORES)

    xT_d = nc.dram_tensor("xT", [U, KL, T], BF16, kind="ExternalInput")
    # chunk-local rotation tables, [n-half-part, k, comp, i]
    Ws_d = [nc.dram_tensor(f"Ws{h}", [128, KL * 2 * TBH], BF16,
                           kind="ExternalInput") for h in range(2)]
    cs_d = [nc.dram_tensor(f"cs{h}", [128, KL * 2 * TBH], BF16,
                           kind="ExternalInput") for h in range(2)]
    co_d = [nc.dram_tensor(f"co{h}", [128, KL * 2 * TBH], BF16,
                           kind="ExternalInput") for h in range(2)]
    coE_d = [nc.dram_tensor(f"coE{h}", [128, KL * 2 * TBH], BF16,
                            kind="ExternalInput") for h in range(2)]
    rm_d = nc.dram_tensor("rmaskA", [128, 2 * KL * 2 * TBH], BF16,
                          kind="ExternalInput")
    # carry re-base constants (hatted system)
    RB1_d = nc.dram_tensor("RB1A", [128, 4 * KL], F32, kind="ExternalInput")
    RB2_d = nc.dram_tensor("RB2A", [128, 4 * KL], F32, kind="ExternalInput")
    # B stationaries: plain taps + per-k a-folded taps
    Bre_d = nc.dram_tensor("BTre", [U, N], BF16, kind="ExternalInput")
    Bim_d = nc.dram_tensor("BTim", [U, N], BF16, kind="ExternalInput")
    B1_d = nc.dram_tensor("B1T", [U, KL, N], BF16, kind="ExternalInput")
    B2_d = nc.dram_tensor("B2T", [U, KL, N], BF16, kind="ExternalInput")
    C1_d = nc.dram_tensor("CT1", [128, N], BF16, kind="ExternalInput")
    C2_d = nc.dram_tensor("CT2", [128, N], BF16, kind="ExternalInput")
    C3_d = nc.dram_tensor("CT3", [128, N], BF16, kind="ExternalInput")
    DT_d = nc.dram_tensor("DT", [U, P], BF16, kind="ExternalInput")
    MDT_d = nc.dram_tensor("MDT", [U, P], BF16, kind="ExternalInput")
    y_d = nc.dram_tensor("yT", [P, KL, T], BF16, kind="ExternalOutput")

    with TileContext(nc) as tc:
        with (
            tc.tile_pool(name="const", bufs=1) as cpool,
            tc.tile_pool(name="xp", bufs=2) as xpool,
            tc.tile_pool(name="wa", bufs=2) as wpool,
            tc.tile_pool(name="pp", bufs=1) as ppool,
            tc.tile_pool(name="uh", bufs=2) as uhpool,
            tc.tile_pool(name="qq", bufs=2) as qpool,
            tc.tile_pool(name="pr", bufs=2) as prpool,
            tc.tile_pool(name="rz", bufs=2) as rzpool,
            tc.tile_pool(name="yo", bufs=2) as ypool,
            tc.tile_pool(name="wps", bufs=2, space="PSUM") as wpsum,
            tc.tile_pool(name="yps", bufs=1, space="PSUM") as ypsum,
        ):
            Bre = cpool.tile([U, N], BF16)
            nc.sync.dma_start(Bre[:], Bre_d[:])
            Bim = cpool.tile([U, N], BF16)
            nc.sync.dma_start(Bim[:], Bim_d[:])
            B1 = cpool.tile([U, KL, N], BF16)
            nc.sync.dma_start(B1[:], B1_d[:])
            B2 = cpool.tile([U, KL, N], BF16)
            nc.sync.dma_start(B2[:], B2_d[:])
            # chunk-0 x prefetch right after the B stationaries so the
            # B matmuls start while the big tables stream in.
            xt0 = xpool.tile([U, KL, TBH, 2], BF16, tag="x", name="xt0")
            nc.sync.dma_start(
                xt0[:].rearrange("u k m q -> u k (m q)"), xT_d[:, :, 0:TB])
            Ws, cs3, csO, csOE = [], [], [], []
            for h in range(2):
                t2 = cpool.tile([128, KL, 2, TBH], BF16, name=f"Wst{h}",
                                tag=f"Wst{h}")
                nc.sync.dma_start(
                    t2[:].rearrange("p k c t -> p (k c t)"), Ws_d[h][:])
                Ws.append(t2)
                t3 = cpool.tile([128, KL, 2, TBH], BF16, name=f"cst{h}",
                                tag=f"cst{h}")
                nc.sync.dma_start(
                    t3[:].rearrange("p k c t -> p (k c t)"), cs_d[h][:])
                cs3.append(t3)
            rmaskA = cpool.tile([128, 2 * KL * 2 * TBH], BF16,
                                name="rmaskA", tag="rmaskA")
            nc.sync.dma_start(rmaskA[:], rm_d[:])
            for h in range(2):
                t3o = cpool.tile([128, KL, 2, TBH], BF16, name=f"csot{h}",
                                 tag=f"csot{h}")
                nc.sync.dma_start(
                    t3o[:].rearrange("p k c t -> p (k c t)"), co_d[h][:])
                csO.append(t3o)
                t3e = cpool.tile([128, KL, 2, TBH], BF16, name=f"csoEt{h}",
                                 tag=f"csoEt{h}")
                nc.sync.dma_start(
                    t3e[:].rearrange("p k c t -> p (k c t)"), coE_d[h][:])
                csOE.append(t3e)
            C1 = cpool.tile([128, N], BF16)
            nc.sync.dma_start(C1[:], C1_d[:])
            C2 = cpool.tile([128, N], BF16)
            nc.sync.dma_start(C2[:], C2_d[:])
            C3 = cpool.tile([128, N], BF16)
            nc.sync.dma_start(C3[:], C3_d[:])
            DT = cpool.tile([U, P], BF16)
            nc.sync.dma_start(DT[:], DT_d[:])
            MDT = cpool.tile([U, P], BF16)
            nc.sync.dma_start(MDT[:], MDT_d[:])
            RB1A = cpool.tile([128, 4 * KL], F32, name="RB1A", tag="RB1A")
            nc.sync.dma_start(RB1A[:], RB1_d[:])
            RB2A = cpool.tile([128, 4 * KL], F32, name="RB2A", tag="RB2A")
            nc.sync.dma_start(RB2A[:], RB2_d[:])

            def b2k(ap):
                # [128, KL, 1, TBH] slice -> [128, KL, 2, TBH] stride-0 pair
                return ap.broadcast_to([128, KL, 2, TBH])

            rz_prev = None
            prE_prev = None
            for tb in range(NT):
                t0 = tb * TB
                if tb == 0:
                    xt = xt0
                else:
                    xt = xpool.tile([U, KL, TBH, 2], BF16, tag="x")
                    nc.sync.dma_start(
                        xt[:].rearrange("u k m q -> u k (m q)"),
                        xT_d[:, :, t0:t0 + TB])

                # ---- B matmuls: w = B1@x_e + B@x_o  (PSUM accumulate) ----
                w_all = [wpool.tile([128, KL, 2, TBH], BF16, tag=f"w{h}",
                                    name=f"wall{h}")
                         for h in range(2)]
                for h in range(2):
                    hs = slice(h * 128, (h + 1) * 128)
                    for k in range(KL):
                        w_ps = wpsum.tile([128, 2, TBH], F32, tag="w")
                        nc.tensor.matmul(w_ps[:, 0, :], B1[:, k, hs],
                                         xt[:, k, :, 0],
                                         start=True, stop=False)
                        nc.tensor.matmul(w_ps[:, 0, :], Bre[:, hs],
                                         xt[:, k, :, 1],
                                         start=False, stop=True)
                        nc.tensor.matmul(w_ps[:, 1, :], B2[:, k, hs],
                                         xt[:, k, :, 0],
                                         start=True, stop=False)
                        nc.tensor.matmul(w_ps[:, 1, :], Bim[:, hs],
                                         xt[:, k, :, 1],
                                         start=False, stop=True)
                        nc.scalar.copy(w_all[h][:, k], w_ps[:])

                # ---- rotate-in (DVE, bf16 2x) ----
                uhall = uhpool.tile([128, 2, KL, 2, TBH], BF16, tag="uh")
                for h in range(2):
                    pA = ppool.tile([128, KL, 2, TBH], BF16, tag=f"pA{h}")
                    nc.vector.tensor_mul(pA[:], cs3[h][:],
                                         b2k(w_all[h][:, :, 0:1, :]))
                    pB = ppool.tile([128, KL, 2, TBH], BF16, tag=f"pB{h}")
                    nc.vector.tensor_mul(pB[:], Ws[h][:],
                                         b2k(w_all[h][:, :, 1:2, :]))
                    # uh[c0] = cl*gi*w_re + sl*gi*w_im ; uh[c1] = -gi*Im(..)
                    nc.vector.tensor_sub(uhall[:, h], pA[:], pB[:])
                # inject carried state into each segment's first column
                if tb > 0:
                    rz4 = rz_prev[:].rearrange("p (h k c) -> p h k c", h=2,
                                               k=KL)
                    nc.vector.tensor_add(uhall[:, :, :, :, 0],
                                         uhall[:, :, :, :, 0], rz4[:])

                # ---- scan (DVE, fp32 state, bf16 in/out) ----
                qA = qpool.tile([128, 2, KL, 2, TBH], BF16, tag="q",
                                name="qA")
                nc.vector.tensor_tensor_scan(
                    qA[:].rearrange("p h k c t -> p (h k c t)"),
                    rmaskA[:],
                    uhall[:].rearrange("p h k c t -> p (h k c t)"),
                    0.0, mult, add)

                # ---- carry re-base for next chunk ----
                if tb + 1 < NT:
                    zq = rzpool.tile([128, 4 * KL], F32, tag="zq")
                    nc.scalar.copy(
                        zq[:].rearrange("p (h k c) -> p h k c", h=2, k=KL),
                        qA[:, :, :, :, TBH - 1])
                    m1 = rzpool.tile([128, 4 * KL], F32, tag="m1")
                    nc.vector.tensor_mul(m1[:], zq[:], RB1A[:])
                    m2 = rzpool.tile([128, 4 * KL], F32, tag="m2")
                    zq4 = zq[:].rearrange("p (h k c) -> p h k c", h=2, k=KL)
                    m24 = m2[:].rearrange("p (h k c) -> p h k c", h=2, k=KL)
                    rb4 = RB2A[:].rearrange("p (h k c) -> p h k c", h=2,
                                            k=KL)
                    nc.vector.tensor_mul(m24[:, :, :, 0:1], zq4[:, :, :, 1:2],
                                         rb4[:, :, :, 0:1])
                    nc.vector.tensor_mul(m24[:, :, :, 1:2], zq4[:, :, :, 0:1],
                                         rb4[:, :, :, 1:2])
                    rzt = rzpool.tile([128, 4 * KL], F32, tag="rz")
                    nc.vector.tensor_add(rzt[:], m1[:], m2[:])
                    rz_prev = rzt

                # ---- rotate-out products (DVE) ----
                # odd set (csO): cols 0..TBH-1 = sigma[m0+i] products
                # even set (csOE, a-premultiplied): written at col offset 1;
                # col 0 = previous chunk's last col (a*sigma_end carry).
                pt, qt, ptE, qtE = [], [], [], []
                for h in range(2):
                    p1 = prpool.tile([128, KL, 2, TBH], BF16, tag=f"pt{h}")
                    nc.vector.tensor_mul(p1[:], csO[h][:],
                                         b2k(qA[:, h, :, 0:1, :]))
                    pt.append(p1)
                    q1 = prpool.tile([128, KL, 2, TBH], BF16, tag=f"qt{h}")
                    nc.vector.tensor_mul(q1[:], csO[h][:],
                                         b2k(qA[:, h, :, 1:2, :]))
                    qt.append(q1)
                    p2 = prpool.tile([128, KL, 2, TBH + 1], BF16,
                                     tag=f"ptE{h}")
                    if tb == 0:
                        nc.vector.memset(p2[:, :, :, 0:1], 0.0)
                    else:
                        nc.scalar.copy(p2[:, :, :, 0:1],
                                       prE_prev[0][h][:, :, :, TBH:TBH + 1])
                    nc.vector.tensor_mul(p2[:, :, :, 1:TBH + 1], csOE[h][:],
                                         b2k(qA[:, h, :, 0:1, :]))
                    ptE.append(p2)
                    q2 = prpool.tile([128, KL, 2, TBH + 1], BF16,
                                     tag=f"qtE{h}")
                    if tb == 0:
                        nc.vector.memset(q2[:, :, :, 0:1], 0.0)
                    else:
                        nc.scalar.copy(q2[:, :, :, 0:1],
                                       prE_prev[1][h][:, :, :, TBH:TBH + 1])
                    nc.vector.tensor_mul(q2[:, :, :, 1:TBH + 1], csOE[h][:],
                                         b2k(qA[:, h, :, 1:2, :]))
                    qtE.append(q2)
                prE_prev = (ptE, qtE)

                # ---- C/D matmuls into 4 PSUM tiles (odd/even x kpair) ----
                yps = {("o", 0): ypsum.tile([128, 2, TBH], F32, tag="yo0",
                                            name="ypso0"),
                       ("o", 1): ypsum.tile([128, 2, TBH], F32, tag="yo1",
                                            name="ypso1"),
                       ("e", 0): ypsum.tile([128, 2, TBH], F32, tag="ye0",
                                            name="ypse0"),
                       ("e", 1): ypsum.tile([128, 2, TBH], F32, tag="ye1",
                                            name="ypse1")}
                fams = [(C1, 0, "p", 0), (C3, 0, "p", 1),
                        (C1, 1, "p", 0), (C3, 1, "p", 1),
                        (C2, 0, "q", 0), (C1, 0, "q", 1),
                        (C2, 1, "q", 0), (C1, 1, "q", 1)]
                for i, (cst, h, fam, c) in enumerate(fams):
                    hs = slice(h * 128, (h + 1) * 128)
                    for par in ("o", "e"):
                        if par == "o":
                            src = pt[h] if fam == "p" else qt[h]
                        else:
                            src = ptE[h] if fam == "p" else qtE[h]
                        for kp in range(2):
                            kk = slice(2 * kp, 2 * kp + 2)
                            if par == "o":
                                mv = src[:, kk, c, :]
                            else:
                                mv = src[:, kk, c, 0:TBH]
                            nc.tensor.matmul(yps[(par, kp)][:],
                                             cst[:, hs], mv,
                                             start=(i == 0), stop=False)
                # feedthrough taps close each accumulation group
                for kp in range(2):
                    kk = slice(2 * kp, 2 * kp + 2)
                    nc.tensor.matmul(yps[("o", kp)][:], DT[:],
                                     xt[:, kk, :, 1],
                                     start=False, stop=True)
                    nc.tensor.matmul(yps[("e", kp)][:], MDT[:],
                                     xt[:, kk, :, 0],
                                     start=False, stop=True)

                # ---- drain + interleaved store ----
                for kp in range(2):
                    y_sb = ypool.tile([128, 2, TBH, 2], BF16, tag="ysb")
                    nc.scalar.copy(y_sb[:, :, :, 0], yps[("e", kp)][:])
                    nc.scalar.copy(y_sb[:, :, :, 1], yps[("o", kp)][:])
                    nc.sync.dma_start(
                        y_d[:, 2 * kp:2 * kp + 2, t0:t0 + TB],
                        y_sb[:].rearrange("p k m q -> p k (m q)"))

    nc.compile()
    return nc


def _host_prep(input_sequence, A_re, A_im, B_re, B_im, C_re, C_im, D):
    """Build the per-core input maps (numpy only)."""
    x = np.ascontiguousarray(np.asarray(input_sequence), dtype=np.float32)
    A_re = np.asarray(A_re, dtype=np.float32)
    A_im = np.asarray(A_im, dtype=np.float32)
    B_re = np.asarray(B_re, dtype=np.float32)
    B_im = np.asarray(B_im, dtype=np.float32)
    C_re = np.asarray(C_re, dtype=np.float32)
    C_im = np.asarray(C_im, dtype=np.float32)
    D = np.asarray(D, dtype=np.float32)

    th = np.arctan2(A_im.astype(np.float64), A_re.astype(np.float64))  # (N,K)
    r = np.hypot(A_re.astype(np.float64), A_im.astype(np.float64))    # (N,K)
    thh = 2.0 * th
    rh = r * r
    rht = rh.astype(np.float32).astype(BF16NP).astype(np.float64)  # exact

    i = np.arange(TBH, dtype=np.float64)

    BTre = np.ascontiguousarray(B_re.T).astype(BF16NP)      # (U, N)
    BTim = np.ascontiguousarray(B_im.T).astype(BF16NP)
    a_re = (r * np.cos(th))
    a_im = (r * np.sin(th))
    CT1 = np.concatenate([C_re[:, :128].T, C_re[:, 128:].T], axis=1)
    CT2 = np.concatenate([C_im[:, :128].T, C_im[:, 128:].T], axis=1)
    CT3 = -CT2
    CT1 = np.ascontiguousarray(CT1).astype(BF16NP)          # (128, N)
    CT2 = np.ascontiguousarray(CT2).astype(BF16NP)
    CT3 = np.ascontiguousarray(CT3).astype(BF16NP)
    DTm = np.ascontiguousarray(D.T).astype(BF16NP)          # (U, P)
    M = C_re.astype(np.float64) @ B_re.astype(np.float64) \
        - C_im.astype(np.float64) @ B_im.astype(np.float64)
    MDT = np.ascontiguousarray((M + D).T).astype(BF16NP)    # (U, P)

    in_maps = []
    for cidx in range(NCORES):
        ks = slice(cidx * KL, (cidx + 1) * KL)
        xT = np.ascontiguousarray(
            x[:, :, ks].transpose(1, 2, 0)).astype(BF16NP)  # (U,KL,T)
        m = dict(xT=xT, BTre=BTre, BTim=BTim, CT1=CT1, CT2=CT2,
                 CT3=CT3, DT=DTm, MDT=MDT)
        # per-k a-folded B taps, transposed: (U, KL, N)
        B1k = (a_re[:, ks, None] * B_re[:, None, :]
               - a_im[:, ks, None] * B_im[:, None, :])      # (N, KL, U)
        B2k = (a_re[:, ks, None] * B_im[:, None, :]
               + a_im[:, ks, None] * B_re[:, None, :])
        m["B1T"] = np.ascontiguousarray(
            B1k.transpose(2, 1, 0)).astype(BF16NP)
        m["B2T"] = np.ascontiguousarray(
            B2k.transpose(2, 1, 0)).astype(BF16NP)
        for h in range(2):
            hs = slice(h * 128, (h + 1) * 128)
            thl = thh[hs, ks]                                # (128, KL)
            rhl = rh[hs, ks]
            rtl = rht[hs, ks]
            ang = thl[:, :, None] * i[None, None, :]         # (128, KL, TBH)
            cl = np.cos(ang)
            sl = np.sin(ang)
            g = np.exp(np.log(rhl / rtl)[:, :, None] * i)    # (128, KL, TBH)
            gi = 1.0 / g
            ws = np.stack([-sl * gi, cl * gi], axis=2)
            c3 = np.stack([cl * gi, sl * gi], axis=2)
            cO = np.stack([cl * g, sl * g], axis=2)
            phi = ang + th[hs, ks][:, :, None]
            rl = r[hs, ks][:, :, None]
            cOE = np.stack([rl * np.cos(phi) * g,
                            rl * np.sin(phi) * g], axis=2)
            m[f"Ws{h}"] = np.ascontiguousarray(
                ws.reshape(128, -1)).astype(BF16NP)
            m[f"cs{h}"] = np.ascontiguousarray(
                c3.reshape(128, -1)).astype(BF16NP)
            m[f"co{h}"] = np.ascontiguousarray(
                cO.reshape(128, -1)).astype(BF16NP)
            m[f"coE{h}"] = np.ascontiguousarray(
                cOE.reshape(128, -1)).astype(BF16NP)
            # scan multiplier mask: rht everywhere, 0 at segment starts
            rmk = np.broadcast_to(
                rtl.astype(np.float32).astype(BF16NP).astype(np.float64)
                [:, :, None, None], (128, KL, 2, TBH)).copy()
            rmk[:, :, :, 0] = 0.0
            m.setdefault("_rmk", []).append(rmk.reshape(128, -1))
            # carry re-base: inject rho = E''*z,
            # E'' = e^{i thh TBH} * rh^TBH / rht^(TBH-1)
            fac = np.exp(np.log(rhl) * TBH - np.log(rtl) * (TBH - 1))
            Phi = thl * TBH
            cE = np.cos(Phi) * fac                           # (128, KL)
            sE = np.sin(Phi) * fac
            rb1 = np.repeat(cE, 2, axis=1)                   # (128, 2KL)
            m.setdefault("_rb1", []).append(rb1)
            rb2i = np.stack([sE, -sE], axis=2).reshape(128, -1)
            m.setdefault("_rb2", []).append(rb2i)
        m["rmaskA"] = np.ascontiguousarray(
            np.concatenate(m.pop("_rmk"), axis=1)).astype(BF16NP)
        m["RB1A"] = np.ascontiguousarray(
            np.concatenate(m.pop("_rb1"), axis=1)).astype(np.float32)
        m["RB2A"] = np.ascontiguousarray(
            np.concatenate(m.pop("_rb2"), axis=1)).astype(np.float32)
        in_maps.append(m)
    return in_maps


def _get_nc():
    if "nc" not in _CACHE:
        _CACHE["nc"] = _build()
    return _CACHE["nc"]


def kernel(input_sequence, A_re, A_im, B_re, B_im, C_re, C_im, D,
           trace=False):
    nc = _get_nc()
    in_maps = _host_prep(input_sequence, A_re, A_im, B_re, B_im, C_re,
                         C_im, D)
    res = run_bass_kernel_spmd(nc, in_maps, core_ids=list(range(NCORES)),
                               trace=trace)
    out = np.empty((T, P, K), dtype=np.float32)
    for c in range(NCORES):
        yT = res.results[c]["yT"]                    # (P, KL, T) bf16
        out[:, :, c * KL:(c + 1) * KL] = yT.transpose(2, 0, 1) \
            .astype(np.float32)
    if trace:
        _CACHE["exec_time_ns"] = res.exec_time_ns
    return out
